# revision 18
# baseline (speedup 1.0000x reference)
"""Causal multi-head attention on 8 Trainium2 NeuronCores.

Sharding: tensor parallel over heads. Core c owns heads {2c, 2c+1}:
  - QKV projection for its 128 q / 128 k / 128 v channels, all B*S tokens
  - causal attention for its 2 heads (row-packed K=64 matmuls)
  - partial output projection out_c = O_c @ W_o[rows of its heads]
Host side: shard/preprocess inputs (transpose x, slice + pre-scale weights),
then unshard by summing the 8 partial projections (the tensor-parallel
reduce) and adding b_o.

Problem constants (hardcoded per the harness contract):
  x [4, 2048, 1024] f32, W_qkv [1024, 3072], b_qkv [3072],
  W_o [1024, 1024], b_o [1024]; 16 heads, d_k = 64, causal.
"""

import numpy as np

N_CORES = 8
B, S, D = 4, 2048, 1024
H = 16
DK = 64
T = B * S            # 8192 tokens
HPC = H // N_CORES   # 2 heads per core
CPC = HPC * DK       # 128 channels per core per q/k/v
NB = S // 512        # 4 q-chunks of 512 per batch
NK = S // 128        # 16 k-tiles of 128 per batch

_PROFILE = False     # test.py may set kernel._PROFILE = True
_TRACE_DIR = None
_LAST_RESULT = None  # BassKernelResults of the last run (for test.py)

_PROGRAM = None      # cached (nc, input names) across calls


def _build_program():
    import concourse.bacc as bacc
    import concourse.tile as tile
    from concourse import mybir

    F32 = mybir.dt.float32
    BF16 = mybir.dt.bfloat16
    AF = mybir.ActivationFunctionType

    nc = bacc.Bacc("TRN2", num_devices=N_CORES)

    # ---- DRAM parameters (per core) ----
    xT = nc.declare_dram_parameter("xT", [D, T], BF16, isOutput=False)
    wq = nc.declare_dram_parameter("wq", [D, CPC], BF16, isOutput=False)
    wk = nc.declare_dram_parameter("wk", [D, CPC], BF16, isOutput=False)
    wv = nc.declare_dram_parameter("wv", [D, CPC], BF16, isOutput=False)
    bq = nc.declare_dram_parameter("bq", [CPC, 1], F32, isOutput=False)
    bk = nc.declare_dram_parameter("bk", [CPC, 1], F32, isOutput=False)
    bv = nc.declare_dram_parameter("bv", [CPC, 1], F32, isOutput=False)
    wo = nc.declare_dram_parameter("wo", [CPC, D], BF16, isOutput=False)
    gm = nc.declare_dram_parameter("gm", [128, 896], BF16, isOutput=False)
    ident_d = nc.declare_dram_parameter("ident", [128, 128], BF16, isOutput=False)
    ones_d = nc.declare_dram_parameter("ones", [128, 1], BF16, isOutput=False)
    out = nc.declare_dram_parameter("out", [T, D], F32, isOutput=True)

    xT_t = xT.rearrange("(m p) t -> p m t", p=128)      # [128, 8, 8192]
    wq_t = wq.rearrange("(m p) c -> p m c", p=128)      # [128, 8, 128]
    wk_t = wk.rearrange("(m p) c -> p m c", p=128)
    wv_t = wv.rearrange("(m p) c -> p m c", p=128)

    with tile.TileContext(nc) as tc:
        with (
            tc.tile_pool(name="const", bufs=1) as const,
            tc.tile_pool(name="xt", bufs=3) as xt_pool,
            tc.tile_pool(name="qk", bufs=2) as qk_pool,
            tc.tile_pool(name="vt", bufs=3) as vt_pool,
            tc.tile_pool(name="vaug", bufs=2 * NK) as vaug_pool,
            tc.tile_pool(name="pt", bufs=6) as pt_pool,
            tc.tile_pool(name="otn", bufs=2) as otn_pool,
            tc.tile_pool(name="norm", bufs=4) as norm_pool,
            tc.tile_pool(name="osb", bufs=3) as out_pool,
            tc.tile_pool(name="ps", bufs=2, space="PSUM") as ps,
        ):
            # constants
            wq_sb = const.tile([128, 8, CPC], BF16, tag="wq")
            nc.sync.dma_start(wq_sb[:], wq_t)
            wk_sb = const.tile([128, 8, CPC], BF16, tag="wk")
            nc.sync.dma_start(wk_sb[:], wk_t)
            wv_sb = const.tile([128, 8, CPC], BF16, tag="wv")
            nc.sync.dma_start(wv_sb[:], wv_t)
            bq_sb = const.tile([CPC, 1], F32, tag="bq")
            nc.sync.dma_start(bq_sb[:], bq[:])
            bk_sb = const.tile([CPC, 1], F32, tag="bk")
            nc.sync.dma_start(bk_sb[:], bk[:])
            bv_sb = const.tile([CPC, 1], F32, tag="bv")
            nc.sync.dma_start(bv_sb[:], bv[:])
            wo_sb = const.tile([CPC, D], BF16, tag="wo")
            nc.sync.dma_start(wo_sb[:], wo[:])
            g_sb = const.tile([128, 896], BF16, tag="gm")
            nc.sync.dma_start(g_sb[:], gm[:])
            ident = const.tile([128, 128], BF16, tag="ident")
            nc.sync.dma_start(ident[:], ident_d[:])
            ones_sb = const.tile([128, 1], BF16, tag="ones")
            nc.sync.dma_start(ones_sb[:], ones_d[:])

            for b in range(B):
                t0 = b * S

                # ---- phase 1: QKV projection for this batch ----
                qt_sb = qk_pool.tile([CPC, S], BF16, tag="qt")
                kt_sb = qk_pool.tile([CPC, S], BF16, tag="kt")
                vaug = [
                    vaug_pool.tile([128, 2 * (DK + 1)], BF16, tag="vaug",
                                   name=f"vaug_{b}_{j}")
                    for j in range(NK)
                ]
                for ch in range(NB):  # 4 chunks of 512 tokens
                    c0 = ch * 512
                    x_sb = xt_pool.tile([128, 8, 512], BF16, tag="x",
                                        name=f"x_{b}_{ch}")
                    nc.sync.dma_start(x_sb[:], xT_t[:, :, t0 + c0 : t0 + c0 + 512])
                    ps_q = ps.tile([CPC, 512], F32, tag="ps", name="ps_q")
                    ps_k = ps.tile([CPC, 512], F32, tag="ps", name="ps_k")
                    ps_v = ps.tile([CPC, 512], F32, tag="ps", name="ps_v")
                    for w_sb, ps_c in ((wq_sb, ps_q), (wk_sb, ps_k), (wv_sb, ps_v)):
                        for m in range(8):
                            nc.tensor.matmul(ps_c[:], w_sb[:, m, :], x_sb[:, m, :],
                                             start=(m == 0), stop=(m == 7))
                    nc.vector.tensor_scalar_add(qt_sb[:, c0 : c0 + 512], ps_q[:], bq_sb[:])
                    nc.vector.tensor_scalar_add(kt_sb[:, c0 : c0 + 512], ps_k[:], bk_sb[:])
                    # v: evict, then PE-transpose each 128-token block
                    vtmp = vt_pool.tile([CPC, 512], BF16, tag="vtmp",
                                        name=f"vtmp_{b}_{ch}")
                    nc.vector.tensor_scalar_add(vtmp[:], ps_v[:], bv_sb[:])
                    for jj in range(4):
                        j = ch * 4 + jj
                        ps_t = ps.tile([128, 128], BF16, tag="ps", name="ps_t")
                        nc.tensor.transpose(
                            ps_t[:], vtmp[:, jj * 128 : jj * 128 + 128], ident[:])
                        va = vaug[j]
                        # [V_A | 1 | V_B | 1]: one strided copy for both V
                        # halves, one for both ones columns
                        va_g = va.rearrange("p (g c) -> p g c", c=DK + 1)
                        pt_g = ps_t.rearrange("p (g c) -> p g c", c=DK)
                        nc.scalar.copy(va_g[:, :, 0:DK], pt_g[:])
                        nc.vector.tensor_copy(
                            va_g[:, :, DK : DK + 1],
                            ones_sb[:, None, :].to_broadcast([128, 2, 1]))

                # ---- phase 2: attention ----
                otn = otn_pool.tile([128, S], BF16, tag="otn")
                for ch in range(NB):
                    c0 = ch * 512
                    av_a = ps.tile([DK + 1, 512], F32, tag="av", bufs=2, name="av_a")
                    av_b = ps.tile([DK + 1, 512], F32, tag="av", bufs=2, name="av_b")
                    jmax = ch * 4 + 3
                    # per k-tile: one 2-bank score PSUM [head A | head B],
                    # one wide exp covering both heads
                    for j in range(jmax + 1):
                        k0 = j * 128
                        s_ab = ps.tile([128, 2, 512], F32, tag="psw", bufs=2,
                                       name="s_ab")
                        nc.tensor.matmul(
                            s_ab[:, 0, :], kt_sb[0:DK, k0 : k0 + 128],
                            qt_sb[0:DK, c0 : c0 + 512],
                            start=True, stop=True, tile_position=(0, 0),
                        )
                        nc.tensor.matmul(
                            s_ab[:, 1, :], kt_sb[DK:128, k0 : k0 + 128],
                            qt_sb[DK:128, c0 : c0 + 512],
                            start=True, stop=True, tile_position=(64, 0),
                        )
                        p_ab = pt_pool.tile([128, 2, 512], BF16, tag="pt")
                        nc.scalar.activation(p_ab[:], s_ab[:], AF.Exp)
                        r = j - ch * 4
                        if r >= 0:  # diagonal tile: mask (kk <= qq - 128r)
                            mask = g_sb[:, 384 - 128 * r : 896 - 128 * r]
                            mask2 = mask[:, None, :].to_broadcast([128, 2, 512])
                            nc.gpsimd.tensor_mul(p_ab[:], p_ab[:], mask2)
                        first, last = j == 0, j == jmax
                        nc.tensor.matmul(av_a[:], vaug[j][:, 0 : DK + 1],
                                         p_ab[:, 0, :], start=first, stop=last)
                        nc.tensor.matmul(av_b[:], vaug[j][:, DK + 1 : 2 * DK + 2],
                                         p_ab[:, 1, :], start=first, stop=last)
                    # evict raw O^T_aug + denom to SBUF (frees the av PSUM
                    # slots fast), then normalize from SBUF off the PE path
                    for hh, av in ((0, av_a), (1, av_b)):
                        oaug = norm_pool.tile([DK, 512], BF16, tag="oaug")
                        nc.vector.tensor_copy(oaug[:], av[0:DK, :])
                        den = norm_pool.tile([1, 512], F32, tag="den")
                        nc.vector.tensor_copy(den[:], av[DK : DK + 1, :])
                        rec = norm_pool.tile([1, 512], F32, tag="rec")
                        nc.vector.reciprocal_approx_fast(rec[:], den[:])
                        bc = norm_pool.tile([DK, 512], F32, tag="bc")
                        nc.gpsimd.partition_broadcast(bc[:], rec[:])
                        nc.vector.tensor_mul(
                            otn[hh * DK : hh * DK + DK, c0 : c0 + 512],
                            oaug[:], bc[:],
                        )

                # ---- phase 3: partial output projection for this batch ----
                for tt in range(S // 128):  # 16 token tiles
                    q0 = tt * 128
                    o_sb = out_pool.tile([128, D], F32, tag="osb")
                    for half in range(2):
                        n0 = half * 512
                        ps_o = ps.tile([128, 512], F32, tag="av", bufs=2, name="ps_o")
                        nc.tensor.matmul(ps_o[:], otn[:, q0 : q0 + 128],
                                         wo_sb[:, n0 : n0 + 512],
                                         start=True, stop=True)
                        nc.vector.tensor_copy(o_sb[:, n0 : n0 + 512], ps_o[:])
                    nc.sync.dma_start(out[t0 + q0 : t0 + q0 + 128, :], o_sb[:])

    nc.compile()
    return nc


def _get_program():
    global _PROGRAM
    if _PROGRAM is None:
        _PROGRAM = _build_program()
    return _PROGRAM


def kernel(x, W_qkv, b_qkv, W_o, b_o):
    global _LAST_RESULT
    from concourse.bass_utils import run_bass_kernel_spmd

    x = np.asarray(x, np.float32)
    W_qkv = np.asarray(W_qkv, np.float32)
    b_qkv = np.asarray(b_qkv, np.float32)
    W_o = np.asarray(W_o, np.float32)
    b_o = np.asarray(b_o, np.float32)

    # host-side shard/preprocess
    import ml_dtypes
    bf16 = ml_dtypes.bfloat16
    xT = np.ascontiguousarray(x.reshape(T, D).T).astype(bf16)   # [1024, 8192]
    scale = np.float32(1.0 / np.sqrt(DK))
    ident = np.eye(128, dtype=bf16)
    ones = np.ones((128, 1), bf16)
    # G[kk, c] = 1.0 iff kk <= c - 384  (sliding causal mask strip)
    gmask = (np.arange(896)[None, :] - 384 >= np.arange(128)[:, None]).astype(bf16)

    in_maps = []
    for c in range(N_CORES):
        cs = c * CPC
        in_maps.append({
            "xT": xT,
            "wq": np.ascontiguousarray(W_qkv[:, cs : cs + CPC] * scale).astype(bf16),
            "wk": np.ascontiguousarray(W_qkv[:, D + cs : D + cs + CPC]).astype(bf16),
            "wv": np.ascontiguousarray(W_qkv[:, 2 * D + cs : 2 * D + cs + CPC]).astype(bf16),
            "bq": np.ascontiguousarray(b_qkv[cs : cs + CPC, None] * scale),
            "bk": np.ascontiguousarray(b_qkv[D + cs : D + cs + CPC, None]),
            "bv": np.ascontiguousarray(b_qkv[2 * D + cs : 2 * D + cs + CPC, None]),
            "wo": np.ascontiguousarray(W_o[cs : cs + CPC, :]).astype(bf16),
            "gm": gmask,
            "ident": ident,
            "ones": ones,
        })

    nc = _get_program()
    res = run_bass_kernel_spmd(
        nc, in_maps, list(range(N_CORES)),
        trace=_PROFILE, tmpdir=_TRACE_DIR,
    )
    _LAST_RESULT = res

    # unshard: tensor-parallel reduce of the 8 partial projections + b_o
    acc = res.results[0]["out"].astype(np.float32)
    for c in range(1, N_CORES):
        acc += res.results[c]["out"]
    acc += b_o[None, :]
    return acc.reshape(B, S, D)


# revision 19
# speedup vs baseline: 1.3637x; 1.3637x over previous
"""Causal multi-head attention on 8 Trainium2 NeuronCores.

Sharding: tensor parallel over heads. Core c owns heads {2c, 2c+1}:
  - QKV projection for its 128 q / 128 k / 128 v channels, all B*S tokens
  - causal attention for its 2 heads (row-packed K=64 matmuls)
  - partial output projection out_c = O_c @ W_o[rows of its heads]
Host side: shard/preprocess inputs (transpose x, slice + pre-scale weights),
then unshard by summing the 8 partial projections (the tensor-parallel
reduce) and adding b_o.

Problem constants (hardcoded per the harness contract):
  x [4, 2048, 1024] f32, W_qkv [1024, 3072], b_qkv [3072],
  W_o [1024, 1024], b_o [1024]; 16 heads, d_k = 64, causal.
"""

import numpy as np

N_CORES = 8
B, S, D = 4, 2048, 1024
H = 16
DK = 64
T = B * S            # 8192 tokens
HPC = H // N_CORES   # 2 heads per core
CPC = HPC * DK       # 128 channels per core per q/k/v
NB = S // 512        # 4 q-chunks of 512 per batch
NK = S // 128        # 16 k-tiles of 128 per batch

_PROFILE = False     # test.py may set kernel._PROFILE = True
_TRACE_DIR = None
_LAST_RESULT = None  # BassKernelResults of the last run (for test.py)

_PROGRAM = None      # cached (nc, input names) across calls


def _build_program():
    import concourse.bacc as bacc
    import concourse.tile as tile
    from concourse import mybir

    F32 = mybir.dt.float32
    BF16 = mybir.dt.bfloat16
    AF = mybir.ActivationFunctionType

    nc = bacc.Bacc("TRN2", num_devices=N_CORES)

    # ---- DRAM parameters (per core) ----
    xT = nc.declare_dram_parameter("xT", [D, T], BF16, isOutput=False)
    wq = nc.declare_dram_parameter("wq", [D, CPC], BF16, isOutput=False)
    wk = nc.declare_dram_parameter("wk", [D, CPC], BF16, isOutput=False)
    wv = nc.declare_dram_parameter("wv", [D, CPC], BF16, isOutput=False)
    bq = nc.declare_dram_parameter("bq", [CPC, 1], F32, isOutput=False)
    bk = nc.declare_dram_parameter("bk", [CPC, 1], F32, isOutput=False)
    bv = nc.declare_dram_parameter("bv", [CPC, 1], F32, isOutput=False)
    wo = nc.declare_dram_parameter("wo", [CPC, D], BF16, isOutput=False)
    gm = nc.declare_dram_parameter("gm", [128, 896], BF16, isOutput=False)
    ident_d = nc.declare_dram_parameter("ident", [128, 128], BF16, isOutput=False)
    ones_d = nc.declare_dram_parameter("ones", [128, 1], BF16, isOutput=False)
    out = nc.declare_dram_parameter("out", [T, D], F32, isOutput=True)

    xT_t = xT.rearrange("(m p) t -> p m t", p=128)      # [128, 8, 8192]
    wq_t = wq.rearrange("(m p) c -> p m c", p=128)      # [128, 8, 128]
    wk_t = wk.rearrange("(m p) c -> p m c", p=128)
    wv_t = wv.rearrange("(m p) c -> p m c", p=128)

    with tile.TileContext(nc) as tc:
        with (
            tc.tile_pool(name="const", bufs=1) as const,
            tc.tile_pool(name="xt", bufs=3) as xt_pool,
            tc.tile_pool(name="qk", bufs=2) as qk_pool,
            tc.tile_pool(name="vt", bufs=3) as vt_pool,
            tc.tile_pool(name="vaug", bufs=2 * NK) as vaug_pool,
            tc.tile_pool(name="pt", bufs=6) as pt_pool,
            tc.tile_pool(name="otn", bufs=2) as otn_pool,
            tc.tile_pool(name="norm", bufs=4) as norm_pool,
            tc.tile_pool(name="osb", bufs=3) as out_pool,
            tc.tile_pool(name="ps", bufs=2, space="PSUM") as ps,
        ):
            # constants
            wq_sb = const.tile([128, 8, CPC], BF16, tag="wq")
            nc.sync.dma_start(wq_sb[:], wq_t)
            wk_sb = const.tile([128, 8, CPC], BF16, tag="wk")
            nc.sync.dma_start(wk_sb[:], wk_t)
            wv_sb = const.tile([128, 8, CPC], BF16, tag="wv")
            nc.sync.dma_start(wv_sb[:], wv_t)
            bq_sb = const.tile([CPC, 1], F32, tag="bq")
            nc.sync.dma_start(bq_sb[:], bq[:])
            bk_sb = const.tile([CPC, 1], F32, tag="bk")
            nc.sync.dma_start(bk_sb[:], bk[:])
            bv_sb = const.tile([CPC, 1], F32, tag="bv")
            nc.sync.dma_start(bv_sb[:], bv[:])
            wo_sb = const.tile([CPC, D], BF16, tag="wo")
            nc.sync.dma_start(wo_sb[:], wo[:])
            g_sb = const.tile([128, 896], BF16, tag="gm")
            nc.sync.dma_start(g_sb[:], gm[:])
            ident = const.tile([128, 128], BF16, tag="ident")
            nc.sync.dma_start(ident[:], ident_d[:])
            ones_sb = const.tile([128, 1], BF16, tag="ones")
            nc.sync.dma_start(ones_sb[:], ones_d[:])

            for b in range(B):
                t0 = b * S

                # ---- phase 1: QKV projection for this batch ----
                qt_sb = qk_pool.tile([CPC, S], BF16, tag="qt")
                kt_sb = qk_pool.tile([CPC, S], BF16, tag="kt")
                vaug = [
                    vaug_pool.tile([128, 2 * (DK + 1)], BF16, tag="vaug",
                                   name=f"vaug_{b}_{j}")
                    for j in range(NK)
                ]
                for ch in range(NB):  # 4 chunks of 512 tokens
                    c0 = ch * 512
                    x_sb = xt_pool.tile([128, 8, 512], BF16, tag="x",
                                        name=f"x_{b}_{ch}")
                    nc.sync.dma_start(x_sb[:], xT_t[:, :, t0 + c0 : t0 + c0 + 512])
                    ps_q = ps.tile([CPC, 512], F32, tag="ps", name="ps_q")
                    ps_k = ps.tile([CPC, 512], F32, tag="ps", name="ps_k")
                    ps_v = ps.tile([CPC, 512], F32, tag="ps", name="ps_v")
                    for w_sb, ps_c in ((wq_sb, ps_q), (wk_sb, ps_k), (wv_sb, ps_v)):
                        for m in range(8):
                            nc.tensor.matmul(ps_c[:], w_sb[:, m, :], x_sb[:, m, :],
                                             start=(m == 0), stop=(m == 7))
                    nc.vector.tensor_scalar_add(qt_sb[:, c0 : c0 + 512], ps_q[:], bq_sb[:])
                    nc.vector.tensor_scalar_add(kt_sb[:, c0 : c0 + 512], ps_k[:], bk_sb[:])
                    # v: evict, then PE-transpose each 128-token block
                    vtmp = vt_pool.tile([CPC, 512], BF16, tag="vtmp",
                                        name=f"vtmp_{b}_{ch}")
                    nc.vector.tensor_scalar_add(vtmp[:], ps_v[:], bv_sb[:])
                    for jj in range(4):
                        j = ch * 4 + jj
                        ps_t = ps.tile([128, 128], BF16, tag="ps", name="ps_t")
                        nc.tensor.transpose(
                            ps_t[:], vtmp[:, jj * 128 : jj * 128 + 128], ident[:])
                        va = vaug[j]
                        # [V_A | 1 | V_B | 1]: one strided copy for both V
                        # halves, one for both ones columns
                        va_g = va.rearrange("p (g c) -> p g c", c=DK + 1)
                        pt_g = ps_t.rearrange("p (g c) -> p g c", c=DK)
                        nc.scalar.copy(va_g[:, :, 0:DK], pt_g[:])
                        nc.vector.tensor_copy(
                            va_g[:, :, DK : DK + 1],
                            ones_sb[:, None, :].to_broadcast([128, 2, 1]))

                # ---- phase 2: attention ----
                otn = otn_pool.tile([128, S], BF16, tag="otn")
                for ch in range(NB):
                    c0 = ch * 512
                    av_a = ps.tile([DK + 1, 512], F32, tag="av", bufs=2, name="av_a")
                    av_b = ps.tile([DK + 1, 512], F32, tag="av", bufs=2, name="av_b")
                    jmax = ch * 4 + 3
                    # per k-tile: one 2-bank score PSUM [head A | head B],
                    # one wide exp covering both heads
                    for j in range(jmax + 1):
                        k0 = j * 128
                        s_ab = ps.tile([128, 2, 512], F32, tag="psw", bufs=2,
                                       name="s_ab")
                        nc.tensor.matmul(
                            s_ab[:, 0, :], kt_sb[0:DK, k0 : k0 + 128],
                            qt_sb[0:DK, c0 : c0 + 512],
                            start=True, stop=True, tile_position=(0, 0),
                        )
                        nc.tensor.matmul(
                            s_ab[:, 1, :], kt_sb[DK:128, k0 : k0 + 128],
                            qt_sb[DK:128, c0 : c0 + 512],
                            start=True, stop=True, tile_position=(64, 0),
                        )
                        p_ab = pt_pool.tile([128, 2, 512], BF16, tag="pt")
                        nc.scalar.activation(p_ab[:], s_ab[:], AF.Exp)
                        r = j - ch * 4
                        if r >= 0:  # diagonal tile: mask (kk <= qq - 128r)
                            mask = g_sb[:, 384 - 128 * r : 896 - 128 * r]
                            mask2 = mask[:, None, :].to_broadcast([128, 2, 512])
                            nc.vector.tensor_mul(p_ab[:], p_ab[:], mask2)
                        first, last = j == 0, j == jmax
                        nc.tensor.matmul(av_a[:], vaug[j][:, 0 : DK + 1],
                                         p_ab[:, 0, :], start=first, stop=last)
                        nc.tensor.matmul(av_b[:], vaug[j][:, DK + 1 : 2 * DK + 2],
                                         p_ab[:, 1, :], start=first, stop=last)
                    # evict raw O^T_aug + denom to SBUF (frees the av PSUM
                    # slots fast), then normalize from SBUF off the PE path
                    for hh, av in ((0, av_a), (1, av_b)):
                        oaug = norm_pool.tile([DK, 512], BF16, tag="oaug")
                        nc.vector.tensor_copy(oaug[:], av[0:DK, :])
                        den = norm_pool.tile([1, 512], F32, tag="den")
                        nc.vector.tensor_copy(den[:], av[DK : DK + 1, :])
                        rec = norm_pool.tile([1, 512], F32, tag="rec")
                        nc.vector.reciprocal_approx_fast(rec[:], den[:])
                        bc = norm_pool.tile([DK, 512], F32, tag="bc")
                        nc.gpsimd.partition_broadcast(bc[:], rec[:])
                        nc.vector.tensor_mul(
                            otn[hh * DK : hh * DK + DK, c0 : c0 + 512],
                            oaug[:], bc[:],
                        )

                # ---- phase 3: partial output projection for this batch ----
                for tt in range(S // 128):  # 16 token tiles
                    q0 = tt * 128
                    o_sb = out_pool.tile([128, D], F32, tag="osb")
                    for half in range(2):
                        n0 = half * 512
                        ps_o = ps.tile([128, 512], F32, tag="av", bufs=2, name="ps_o")
                        nc.tensor.matmul(ps_o[:], otn[:, q0 : q0 + 128],
                                         wo_sb[:, n0 : n0 + 512],
                                         start=True, stop=True)
                        nc.vector.tensor_copy(o_sb[:, n0 : n0 + 512], ps_o[:])
                    nc.sync.dma_start(out[t0 + q0 : t0 + q0 + 128, :], o_sb[:])

    nc.compile()
    return nc


def _get_program():
    global _PROGRAM
    if _PROGRAM is None:
        _PROGRAM = _build_program()
    return _PROGRAM


def kernel(x, W_qkv, b_qkv, W_o, b_o):
    global _LAST_RESULT
    from concourse.bass_utils import run_bass_kernel_spmd

    x = np.asarray(x, np.float32)
    W_qkv = np.asarray(W_qkv, np.float32)
    b_qkv = np.asarray(b_qkv, np.float32)
    W_o = np.asarray(W_o, np.float32)
    b_o = np.asarray(b_o, np.float32)

    # host-side shard/preprocess
    import ml_dtypes
    bf16 = ml_dtypes.bfloat16
    xT = np.ascontiguousarray(x.reshape(T, D).T).astype(bf16)   # [1024, 8192]
    scale = np.float32(1.0 / np.sqrt(DK))
    ident = np.eye(128, dtype=bf16)
    ones = np.ones((128, 1), bf16)
    # G[kk, c] = 1.0 iff kk <= c - 384  (sliding causal mask strip)
    gmask = (np.arange(896)[None, :] - 384 >= np.arange(128)[:, None]).astype(bf16)

    in_maps = []
    for c in range(N_CORES):
        cs = c * CPC
        in_maps.append({
            "xT": xT,
            "wq": np.ascontiguousarray(W_qkv[:, cs : cs + CPC] * scale).astype(bf16),
            "wk": np.ascontiguousarray(W_qkv[:, D + cs : D + cs + CPC]).astype(bf16),
            "wv": np.ascontiguousarray(W_qkv[:, 2 * D + cs : 2 * D + cs + CPC]).astype(bf16),
            "bq": np.ascontiguousarray(b_qkv[cs : cs + CPC, None] * scale),
            "bk": np.ascontiguousarray(b_qkv[D + cs : D + cs + CPC, None]),
            "bv": np.ascontiguousarray(b_qkv[2 * D + cs : 2 * D + cs + CPC, None]),
            "wo": np.ascontiguousarray(W_o[cs : cs + CPC, :]).astype(bf16),
            "gm": gmask,
            "ident": ident,
            "ones": ones,
        })

    nc = _get_program()
    res = run_bass_kernel_spmd(
        nc, in_maps, list(range(N_CORES)),
        trace=_PROFILE, tmpdir=_TRACE_DIR,
    )
    _LAST_RESULT = res

    # unshard: tensor-parallel reduce of the 8 partial projections + b_o
    acc = res.results[0]["out"].astype(np.float32)
    for c in range(1, N_CORES):
        acc += res.results[c]["out"]
    acc += b_o[None, :]
    return acc.reshape(B, S, D)


# revision 20
# speedup vs baseline: 1.4028x; 1.0286x over previous
"""Causal multi-head attention on 8 Trainium2 NeuronCores.

Sharding: tensor parallel over heads. Core c owns heads {2c, 2c+1}:
  - QKV projection for its 128 q / 128 k / 128 v channels, all B*S tokens
  - causal attention for its 2 heads (row-packed K=64 matmuls)
  - partial output projection out_c = O_c @ W_o[rows of its heads]
Host side: shard/preprocess inputs (transpose x, slice + pre-scale weights),
then unshard by summing the 8 partial projections (the tensor-parallel
reduce) and adding b_o.

Problem constants (hardcoded per the harness contract):
  x [4, 2048, 1024] f32, W_qkv [1024, 3072], b_qkv [3072],
  W_o [1024, 1024], b_o [1024]; 16 heads, d_k = 64, causal.
"""

import numpy as np

N_CORES = 8
B, S, D = 4, 2048, 1024
H = 16
DK = 64
T = B * S            # 8192 tokens
HPC = H // N_CORES   # 2 heads per core
CPC = HPC * DK       # 128 channels per core per q/k/v
NB = S // 512        # 4 q-chunks of 512 per batch
NK = S // 128        # 16 k-tiles of 128 per batch

_PROFILE = False     # test.py may set kernel._PROFILE = True
_TRACE_DIR = None
_LAST_RESULT = None  # BassKernelResults of the last run (for test.py)

_PROGRAM = None      # cached (nc, input names) across calls


def _build_program():
    import concourse.bacc as bacc
    import concourse.tile as tile
    from concourse import mybir

    F32 = mybir.dt.float32
    BF16 = mybir.dt.bfloat16
    AF = mybir.ActivationFunctionType

    nc = bacc.Bacc("TRN2", num_devices=N_CORES)

    # ---- DRAM parameters (per core) ----
    xT = nc.declare_dram_parameter("xT", [D, T], BF16, isOutput=False)
    wq = nc.declare_dram_parameter("wq", [D, CPC], BF16, isOutput=False)
    wk = nc.declare_dram_parameter("wk", [D, CPC], BF16, isOutput=False)
    wv = nc.declare_dram_parameter("wv", [D, CPC], BF16, isOutput=False)
    bq = nc.declare_dram_parameter("bq", [CPC, 1], F32, isOutput=False)
    bk = nc.declare_dram_parameter("bk", [CPC, 1], F32, isOutput=False)
    bv = nc.declare_dram_parameter("bv", [CPC, 1], F32, isOutput=False)
    wo = nc.declare_dram_parameter("wo", [CPC, D], BF16, isOutput=False)
    gm = nc.declare_dram_parameter("gm", [128, 896], BF16, isOutput=False)
    ident_d = nc.declare_dram_parameter("ident", [128, 128], BF16, isOutput=False)
    ones_d = nc.declare_dram_parameter("ones", [128, 1], BF16, isOutput=False)
    out = nc.declare_dram_parameter("out", [T, D], F32, isOutput=True)

    xT_t = xT.rearrange("(m p) t -> p m t", p=128)      # [128, 8, 8192]
    wq_t = wq.rearrange("(m p) c -> p m c", p=128)      # [128, 8, 128]
    wk_t = wk.rearrange("(m p) c -> p m c", p=128)
    wv_t = wv.rearrange("(m p) c -> p m c", p=128)

    with tile.TileContext(nc) as tc:
        with (
            tc.tile_pool(name="const", bufs=1) as const,
            tc.tile_pool(name="xt", bufs=3) as xt_pool,
            tc.tile_pool(name="qk", bufs=2) as qk_pool,
            tc.tile_pool(name="vt", bufs=3) as vt_pool,
            tc.tile_pool(name="vaug", bufs=2 * NK) as vaug_pool,
            tc.tile_pool(name="pt", bufs=6) as pt_pool,
            tc.tile_pool(name="otn", bufs=2) as otn_pool,
            tc.tile_pool(name="norm", bufs=4) as norm_pool,
            tc.tile_pool(name="osb", bufs=3) as out_pool,
            tc.tile_pool(name="ps", bufs=2, space="PSUM") as ps,
        ):
            # constants
            wq_sb = const.tile([128, 8, CPC], BF16, tag="wq")
            nc.sync.dma_start(wq_sb[:], wq_t)
            wk_sb = const.tile([128, 8, CPC], BF16, tag="wk")
            nc.sync.dma_start(wk_sb[:], wk_t)
            wv_sb = const.tile([128, 8, CPC], BF16, tag="wv")
            nc.sync.dma_start(wv_sb[:], wv_t)
            bq_sb = const.tile([CPC, 1], F32, tag="bq")
            nc.sync.dma_start(bq_sb[:], bq[:])
            bk_sb = const.tile([CPC, 1], F32, tag="bk")
            nc.sync.dma_start(bk_sb[:], bk[:])
            bv_sb = const.tile([CPC, 1], F32, tag="bv")
            nc.sync.dma_start(bv_sb[:], bv[:])
            wo_sb = const.tile([CPC, D], BF16, tag="wo")
            nc.sync.dma_start(wo_sb[:], wo[:])
            g_sb = const.tile([128, 896], BF16, tag="gm")
            nc.sync.dma_start(g_sb[:], gm[:])
            ident = const.tile([128, 128], BF16, tag="ident")
            nc.sync.dma_start(ident[:], ident_d[:])
            ones_sb = const.tile([128, 1], BF16, tag="ones")
            nc.sync.dma_start(ones_sb[:], ones_d[:])

            for b in range(B):
                t0 = b * S

                # ---- phase 1: QKV projection for this batch ----
                qt_sb = qk_pool.tile([CPC, S], BF16, tag="qt")
                kt_sb = qk_pool.tile([CPC, S], BF16, tag="kt")
                vaug = [
                    vaug_pool.tile([128, 2 * (DK + 1)], BF16, tag="vaug",
                                   name=f"vaug_{b}_{j}")
                    for j in range(NK)
                ]
                for ch in range(NB):  # 4 chunks of 512 tokens
                    c0 = ch * 512
                    x_sb = xt_pool.tile([128, 8, 512], BF16, tag="x",
                                        name=f"x_{b}_{ch}")
                    nc.sync.dma_start(x_sb[:], xT_t[:, :, t0 + c0 : t0 + c0 + 512])
                    ps_q = ps.tile([CPC, 512], F32, tag="ps", name="ps_q")
                    ps_k = ps.tile([CPC, 512], F32, tag="ps", name="ps_k")
                    ps_v = ps.tile([CPC, 512], F32, tag="ps", name="ps_v")
                    for w_sb, ps_c in ((wq_sb, ps_q), (wk_sb, ps_k), (wv_sb, ps_v)):
                        for m in range(8):
                            nc.tensor.matmul(ps_c[:], w_sb[:, m, :], x_sb[:, m, :],
                                             start=(m == 0), stop=(m == 7))
                    nc.vector.tensor_scalar_add(qt_sb[:, c0 : c0 + 512], ps_q[:], bq_sb[:])
                    nc.vector.tensor_scalar_add(kt_sb[:, c0 : c0 + 512], ps_k[:], bk_sb[:])
                    # v: evict, then PE-transpose each 128-token block
                    vtmp = vt_pool.tile([CPC, 512], BF16, tag="vtmp",
                                        name=f"vtmp_{b}_{ch}")
                    nc.vector.tensor_scalar_add(vtmp[:], ps_v[:], bv_sb[:])
                    for jj in range(4):
                        j = ch * 4 + jj
                        ps_t = ps.tile([128, 128], BF16, tag="ps", name="ps_t")
                        nc.tensor.transpose(
                            ps_t[:], vtmp[:, jj * 128 : jj * 128 + 128], ident[:])
                        va = vaug[j]
                        # [V_A | 1 | V_B | 1]: one strided copy for both V
                        # halves, one for both ones columns
                        va_g = va.rearrange("p (g c) -> p g c", c=DK + 1)
                        pt_g = ps_t.rearrange("p (g c) -> p g c", c=DK)
                        nc.vector.tensor_copy(va_g[:, :, 0:DK], pt_g[:])
                        nc.vector.tensor_copy(
                            va_g[:, :, DK : DK + 1],
                            ones_sb[:, None, :].to_broadcast([128, 2, 1]))

                # ---- phase 2: attention ----
                otn = otn_pool.tile([128, S], BF16, tag="otn")
                for ch in range(NB):
                    c0 = ch * 512
                    av_a = ps.tile([DK + 1, 512], F32, tag="av", bufs=2, name="av_a")
                    av_b = ps.tile([DK + 1, 512], F32, tag="av", bufs=2, name="av_b")
                    jmax = ch * 4 + 3
                    # per k-tile: one 2-bank score PSUM [head A | head B],
                    # one wide exp covering both heads
                    for j in range(jmax + 1):
                        k0 = j * 128
                        s_ab = ps.tile([128, 2, 512], F32, tag="psw", bufs=2,
                                       name="s_ab")
                        nc.tensor.matmul(
                            s_ab[:, 0, :], kt_sb[0:DK, k0 : k0 + 128],
                            qt_sb[0:DK, c0 : c0 + 512],
                            start=True, stop=True, tile_position=(0, 0),
                        )
                        nc.tensor.matmul(
                            s_ab[:, 1, :], kt_sb[DK:128, k0 : k0 + 128],
                            qt_sb[DK:128, c0 : c0 + 512],
                            start=True, stop=True, tile_position=(64, 0),
                        )
                        p_ab = pt_pool.tile([128, 2, 512], BF16, tag="pt")
                        nc.scalar.activation(p_ab[:], s_ab[:], AF.Exp)
                        r = j - ch * 4
                        if r >= 0:  # diagonal tile: mask (kk <= qq - 128r)
                            mask = g_sb[:, 384 - 128 * r : 896 - 128 * r]
                            mask2 = mask[:, None, :].to_broadcast([128, 2, 512])
                            nc.vector.tensor_mul(p_ab[:], p_ab[:], mask2)
                        first, last = j == 0, j == jmax
                        nc.tensor.matmul(av_a[:], vaug[j][:, 0 : DK + 1],
                                         p_ab[:, 0, :], start=first, stop=last)
                        nc.tensor.matmul(av_b[:], vaug[j][:, DK + 1 : 2 * DK + 2],
                                         p_ab[:, 1, :], start=first, stop=last)
                    # evict raw O^T_aug + denom to SBUF (frees the av PSUM
                    # slots fast), then normalize from SBUF off the PE path
                    for hh, av in ((0, av_a), (1, av_b)):
                        oaug = norm_pool.tile([DK, 512], BF16, tag="oaug")
                        nc.vector.tensor_copy(oaug[:], av[0:DK, :])
                        den = norm_pool.tile([1, 512], F32, tag="den")
                        nc.vector.tensor_copy(den[:], av[DK : DK + 1, :])
                        rec = norm_pool.tile([1, 512], F32, tag="rec")
                        nc.vector.reciprocal_approx_fast(rec[:], den[:])
                        bc = norm_pool.tile([DK, 512], F32, tag="bc")
                        nc.gpsimd.partition_broadcast(bc[:], rec[:])
                        nc.vector.tensor_mul(
                            otn[hh * DK : hh * DK + DK, c0 : c0 + 512],
                            oaug[:], bc[:],
                        )

                # ---- phase 3: partial output projection for this batch ----
                for tt in range(S // 128):  # 16 token tiles
                    q0 = tt * 128
                    o_sb = out_pool.tile([128, D], F32, tag="osb")
                    for half in range(2):
                        n0 = half * 512
                        ps_o = ps.tile([128, 512], F32, tag="av", bufs=2, name="ps_o")
                        nc.tensor.matmul(ps_o[:], otn[:, q0 : q0 + 128],
                                         wo_sb[:, n0 : n0 + 512],
                                         start=True, stop=True)
                        nc.vector.tensor_copy(o_sb[:, n0 : n0 + 512], ps_o[:])
                    nc.sync.dma_start(out[t0 + q0 : t0 + q0 + 128, :], o_sb[:])

    nc.compile()
    return nc


def _get_program():
    global _PROGRAM
    if _PROGRAM is None:
        _PROGRAM = _build_program()
    return _PROGRAM


def kernel(x, W_qkv, b_qkv, W_o, b_o):
    global _LAST_RESULT
    from concourse.bass_utils import run_bass_kernel_spmd

    x = np.asarray(x, np.float32)
    W_qkv = np.asarray(W_qkv, np.float32)
    b_qkv = np.asarray(b_qkv, np.float32)
    W_o = np.asarray(W_o, np.float32)
    b_o = np.asarray(b_o, np.float32)

    # host-side shard/preprocess
    import ml_dtypes
    bf16 = ml_dtypes.bfloat16
    xT = np.ascontiguousarray(x.reshape(T, D).T).astype(bf16)   # [1024, 8192]
    scale = np.float32(1.0 / np.sqrt(DK))
    ident = np.eye(128, dtype=bf16)
    ones = np.ones((128, 1), bf16)
    # G[kk, c] = 1.0 iff kk <= c - 384  (sliding causal mask strip)
    gmask = (np.arange(896)[None, :] - 384 >= np.arange(128)[:, None]).astype(bf16)

    in_maps = []
    for c in range(N_CORES):
        cs = c * CPC
        in_maps.append({
            "xT": xT,
            "wq": np.ascontiguousarray(W_qkv[:, cs : cs + CPC] * scale).astype(bf16),
            "wk": np.ascontiguousarray(W_qkv[:, D + cs : D + cs + CPC]).astype(bf16),
            "wv": np.ascontiguousarray(W_qkv[:, 2 * D + cs : 2 * D + cs + CPC]).astype(bf16),
            "bq": np.ascontiguousarray(b_qkv[cs : cs + CPC, None] * scale),
            "bk": np.ascontiguousarray(b_qkv[D + cs : D + cs + CPC, None]),
            "bv": np.ascontiguousarray(b_qkv[2 * D + cs : 2 * D + cs + CPC, None]),
            "wo": np.ascontiguousarray(W_o[cs : cs + CPC, :]).astype(bf16),
            "gm": gmask,
            "ident": ident,
            "ones": ones,
        })

    nc = _get_program()
    res = run_bass_kernel_spmd(
        nc, in_maps, list(range(N_CORES)),
        trace=_PROFILE, tmpdir=_TRACE_DIR,
    )
    _LAST_RESULT = res

    # unshard: tensor-parallel reduce of the 8 partial projections + b_o
    acc = res.results[0]["out"].astype(np.float32)
    for c in range(1, N_CORES):
        acc += res.results[c]["out"]
    acc += b_o[None, :]
    return acc.reshape(B, S, D)
